# revision 1
# baseline (speedup 1.0000x reference)
"""CapsuleLayer (dynamic routing) Trainium2 kernel.

Math (see reference): u_hat[b,j,n,o] = sum_i x[b,n,i] W[j,n,i,o]; 3 routing
iterations of softmax-over-j (j=2 -> sigmoid of logit diff) + squash.

Design: shard the n axis (91392) over 8 cores. Everything heavy runs as
K=128-tall PE matmuls over host-packed bf16 layouts (full moving ingest):
  - s-type sums  t[b,(j,o)] = sum_{n,i} y[b,n,i] W[j,n,i,o]
       lhsT = y-slice [128n, 16b], rhs = Ws-slice [128n, 32(j,o)],
       4-way PE column-tiling, PSUM-accumulated.
  - logit pass   z[b,n,i] = sum_{j,o} Vt[b,j,o] W[j,n,i,o]
       lhsT = S4 = I_4 (x) Vt^T [128(il,j,o), 64(g,b)],
       rhs = W4 [128(il,j,o), n] per i-half.
    d[b,n] = sum_i x*z via xz elementwise (DVE) + delta-matmul (PE).
  - w = sigmoid(d) i-replicated (ACT, PSUM-broadcast read); y = w*x (DVE).
A pre-sync AllReduce absorbs cross-core launch skew; two 2KB AllReduces
(after s~0 and t1); final partials are gathered on the host.
"""
import sys

sys.path.insert(0, "/opt/trn_rl_repo")

import numpy as np
import ml_dtypes

BF16 = ml_dtypes.bfloat16
N_CORES = 8
B = 16
NIN = 91392
DI = 8
DO = 16
NC_N = NIN // N_CORES  # 11424
EPS = 1e-7

_CACHE = {}


def _ceil_to(v, m):
    return (v + m - 1) // m * m


def host_prep(x, W, n_cores=N_CORES):
    """Split x [B,N,8] / W [2,N,8,16] into per-core packed bf16 layouts."""
    n_per = x.shape[1] // n_cores
    ncp = _ceil_to(n_per, 1024)
    chunks = ncp // 128
    ngrp = chunks // 8  # XW groups of 8 chunks (1024 n)
    oneD = np.zeros((128, 16), dtype=BF16)
    for i in range(8):
        for b in range(16):
            oneD[i * 16 + b, b] = 1.0
    in_maps = []
    for c in range(n_cores):
        xc = np.zeros((B, ncp, DI), dtype=np.float32)
        Wc = np.zeros((2, ncp, DI, DO), dtype=np.float32)
        xc[:, :n_per] = x[:, c * n_per : (c + 1) * n_per]
        Wc[:, :n_per] = W[:, c * n_per : (c + 1) * n_per]
        # xs[n128, (chunk, i, b)] = x[b, n, i]
        xs = (
            xc.reshape(B, chunks, 128, DI)  # b c n i
            .transpose(2, 1, 3, 0)  # n c i b
            .reshape(128, chunks, 128)
        )
        # Ws[n128, (chunk, i, j, o)] = W[j, n, i, o]
        Ws = (
            Wc.reshape(2, chunks, 128, DI, DO)  # j c n i o
            .transpose(2, 1, 3, 0, 4)  # n c i j o
            .reshape(128, chunks, 256)
        )
        # interleave into groups of 8 chunks: [xs(8x128) | ws(8x256)]
        XW = np.empty((128, ngrp, 3072), dtype=BF16)
        xs_g = xs.reshape(128, ngrp, 8 * 128)
        ws_g = Ws.reshape(128, ngrp, 8 * 256)
        XW[:, :, :1024] = xs_g
        XW[:, :, 1024:] = ws_g
        XW = np.ascontiguousarray(XW.reshape(128, ngrp * 3072))
        # W4[(il, j, o), (H, n)] = W[j, n, H*4+il, o]
        W4 = np.ascontiguousarray(
            Wc.reshape(2, ncp, 2, 4, DO)  # j n H il o
            .transpose(3, 0, 4, 2, 1)  # il j o H n
            .reshape(128, 2 * ncp)
        ).astype(BF16)
        # x8[(i, b), n] = x[b, n, i]
        x8 = np.ascontiguousarray(
            xc.transpose(2, 0, 1).reshape(128, ncp)
        ).astype(BF16)
        in_maps.append({"XW": XW, "W4": W4, "x8": x8, "oneD": oneD})
    return in_maps, ncp


def build_kernel(ncp, num_devices=N_CORES):
    from contextlib import ExitStack

    import concourse.bacc as bacc
    import concourse.tile as tile
    from concourse import mybir

    DT = mybir.dt.bfloat16
    F32 = mybir.dt.float32
    AF = mybir.ActivationFunctionType
    chunks = ncp // 128
    zt = ncp // 512
    assert ncp % 1024 == 0
    ngrp = chunks // 8  # XW groups (8 chunks each)
    gz = 4 if zt % 4 == 0 else 1  # z-tiles per W4/x8 load group
    zgrp = zt // gz

    nc = bacc.Bacc(
        "TRN2", target_bir_lowering=False, debug=False, num_devices=num_devices
    )
    xw_in = nc.declare_dram_parameter("XW", [128, ngrp * 3072], DT, isOutput=False)
    w4_in = nc.declare_dram_parameter("W4", [128, 2 * ncp], DT, isOutput=False)
    x8_in = nc.declare_dram_parameter("x8", [128, ncp], DT, isOutput=False)
    oned_in = nc.declare_dram_parameter("oneD", [128, 16], DT, isOutput=False)
    t2_out = nc.declare_dram_parameter("t2", [16, 32], F32, isOutput=True)
    s0g_out = nc.declare_dram_parameter("s0g", [16, 32], F32, isOutput=True)

    ar_bufs = []
    for k in range(3):
        ar_bufs.append(
            (
                nc.dram_tensor(f"ar_in{k}", [16, 32], F32),
                nc.dram_tensor(f"ar_out{k}", [16, 32], F32, addr_space="Shared"),
            )
        )

    with tile.TileContext(nc) as tc, ExitStack() as ctx:
        park = ctx.enter_context(tc.tile_pool(name="park", bufs=1))
        ps_acc = ctx.enter_context(tc.tile_pool(name="ps_acc", bufs=1, space="PSUM"))
        ps_z = ctx.enter_context(tc.tile_pool(name="ps_z", bufs=2, space="PSUM"))
        ps_d = ctx.enter_context(tc.tile_pool(name="ps_d", bufs=3, space="PSUM"))
        work = ctx.enter_context(tc.tile_pool(name="work", bufs=4))
        wpool = ctx.enter_context(tc.tile_pool(name="wpool", bufs=zt))
        small = ctx.enter_context(tc.tile_pool(name="small", bufs=8))

        # ---- pre-sync: absorb cross-core launch skew under the DMA phase ----
        pre_in, pre_out = ar_bufs[2]
        zt_sb = work.tile([16, 32], F32, tag="zt_sb")
        nc.gpsimd.memset(zt_sb[:], 0.0)
        nc.gpsimd.dma_start(pre_in[:], zt_sb[:])
        nc.gpsimd.collective_compute(
            "AllReduce",
            mybir.AluOpType.add,
            replica_groups=[list(range(num_devices))],
            ins=[pre_in[:]],
            outs=[pre_out[:]],
        )

        # ---- resident input tiles ----
        # Spread the bulk loads over all three DMA-capable engine queues
        # (SP / ACT / POOL) — a single queue tops out well under HBM rate.
        dma_engs = [nc.sync, nc.scalar, nc.gpsimd]
        _dma_rr = [0]

        def load(dst_ap, src_ap):
            dma_engs[_dma_rr[0] % 3].dma_start(dst_ap, src_ap)
            _dma_rr[0] += 1

        xw_t = []
        for g in range(ngrp):
            t = park.tile([128, 3072], DT, tag=f"xw{g}")
            load(t[:], xw_in[:, g * 3072 : (g + 1) * 3072])
            xw_t.append(t)
        oneD = park.tile([128, 16], DT, tag="oneD")
        nc.sync.dma_start(oneD[:], oned_in[:])
        w4_t = {0: [], 1: []}
        x8_t = []
        for g in range(zgrp):
            c0, c1 = g * gz * 512, (g + 1) * gz * 512
            for h in (0, 1):
                t = park.tile([128, gz * 512], DT, tag=f"w4_{h}_{g}")
                load(t[:], w4_in[:, h * ncp + c0 : h * ncp + c1])
                w4_t[h].append(t)
            t = park.tile([128, gz * 512], DT, tag=f"x8{g}")
            load(t[:], x8_in[:, c0:c1])
            x8_t.append(t)

        def xs_slice(c, w):
            g, lc = c // 8, c % 8
            return xw_t[g][:, lc * 128 : lc * 128 + w]

        def ws_slice(c, i):
            g, lc = c // 8, c % 8
            off = 1024 + lc * 256 + i * 32
            return xw_t[g][:, off : off + 32]

        def squash(s_tile, scale):
            """v = squash(scale * s), s_tile [16,32] viewed [16,2,16]."""
            sq = small.tile([16, 32], F32, tag="sq")
            nc.vector.tensor_mul(sq[:], s_tile[:], s_tile[:])
            sn = small.tile([16, 2], F32, tag="sn")
            nc.vector.tensor_reduce(
                sn[:],
                sq[:].rearrange("p (j o) -> p j o", j=2),
                mybir.AxisListType.X,
                mybir.AluOpType.add,
            )
            sns = small.tile([16, 2], F32, tag="sns")
            nc.vector.tensor_scalar_mul(sns[:], sn[:], scale * scale)
            den = small.tile([16, 2], F32, tag="den")
            nc.vector.tensor_scalar_add(den, sns[:], 1.0)
            rec = small.tile([16, 2], F32, tag="rec")
            nc.vector.reciprocal(rec[:], den[:])
            epst = small.tile([16, 1], F32, tag="epst")
            nc.vector.memset(epst[:], EPS)
            sr = small.tile([16, 2], F32, tag="sr")
            nc.scalar.activation(sr[:], sns[:], AF.Sqrt, bias=epst[:])
            rs = small.tile([16, 2], F32, tag="rs")
            nc.vector.reciprocal(rs[:], sr[:])
            f = small.tile([16, 2], F32, tag="f")
            nc.vector.tensor_mul(f[:], sns[:], rec[:])
            f2 = small.tile([16, 2], F32, tag="f2")
            nc.vector.tensor_mul(f2[:], f[:], rs[:])
            fs = small.tile([16, 2], F32, tag="fs")
            nc.vector.tensor_scalar_mul(fs[:], f2[:], scale)
            v = small.tile([16, 32], F32, tag=f"v_{scale}_{nc.next_id()}")
            nc.vector.tensor_mul(
                v[:].rearrange("p (j o) -> p j o", j=2),
                s_tile[:].rearrange("p (j o) -> p j o", j=2),
                fs[:].unsqueeze(2).broadcast_to([16, 2, 16]),
            )
            return v

        def all_reduce(src_sb, idx):
            """SBUF [16,32] partial -> SBUF tile of the global sum."""
            a_in, a_out = ar_bufs[idx]
            nc.scalar.dma_start(a_in[:], src_sb[:])
            nc.gpsimd.collective_compute(
                "AllReduce",
                mybir.AluOpType.add,
                replica_groups=[list(range(num_devices))],
                ins=[a_in[:]],
                outs=[a_out[:]],
            )
            g = small.tile([16, 32], F32, tag=f"arg{idx}")
            nc.gpsimd.dma_start(g[:], a_out[:])
            return g

        def s_sweep(lhs_for_chunk, tag):
            """t[b,(j,o)] = sum_{c,i} lhsT^T @ Ws, 4-way PE column-tiled."""
            st_ps = ps_acc.tile([128, 32], F32, tag="stacc")
            nmm = chunks * 8
            for c in range(chunks):
                lhs = lhs_for_chunk(c)
                for i in range(DI):
                    m = c * 8 + i
                    q = m % 4
                    nc.tensor.matmul(
                        st_ps[q * 32 : q * 32 + 16, :],
                        lhs[:, i * 16 : i * 16 + 16],
                        ws_slice(c, i),
                        start=(m < 4),
                        stop=(m >= nmm - 4),
                        tile_position=(0, q * 32),
                        skip_group_check=True,
                    )
            acc = small.tile([16, 32], F32, tag=f"m0_{tag}")
            nc.vector.tensor_copy(acc[:], st_ps[0:16, :])
            for q in (1, 2, 3):
                nxt = small.tile([16, 32], F32, tag=f"m{q}_{tag}")
                nc.vector.tensor_add(nxt[:], acc[:], st_ps[q * 32 : q * 32 + 16, :])
                acc = nxt
            return acc

        # ---- stage A: st0[b,(j,o)] = sum_{n,i} x W ----
        st0_sb = s_sweep(lambda c: xs_slice(c, 128), "a")
        st0g = all_reduce(st0_sb, 0)
        v0 = squash(st0g, 0.5)

        def routing_pass(vacc, it):
            """Given accumulated v [16,32], compute t[b,(j,o)] partial (SBUF)."""
            # Vt transposed + sign: vT[(j,o), b] = +/- vacc[b, (j,o)]
            vt_in = work.tile([32, 32], F32, tag="vt_in")
            nc.vector.memset(vt_in[:], 0.0)
            nc.vector.tensor_copy(vt_in[0:16, 0:16], vacc[:, 0:16])
            nc.scalar.mul(vt_in[0:16, 16:32], vacc[:, 16:32], -1.0)
            vT = work.tile([32, 32], F32, tag="vT")
            nc.vector.transpose(vT[:], vt_in[:])
            # S4 = I_4 (x) vT : [128 (il,j,o), 64 (g,b)]
            s4 = work.tile([128, 64], DT, tag="s4")
            nc.vector.memset(s4[:], 0.0)
            for gg in range(4):
                nc.scalar.copy(
                    s4[gg * 32 : gg * 32 + 32, gg * 16 : gg * 16 + 16],
                    vT[0:32, 0:16],
                )
            # z, d, w per 512-n tile; w batched [128, (csub,i,b)=512] per tile
            wtiles = []
            for t in range(zt):
                zg, off = t // gz, (t % gz) * 512
                z_ps = ps_z.tile([128, 512], F32, tag="z")
                for H in (0, 1):
                    nc.tensor.matmul(
                        z_ps[H * 64 : H * 64 + 64, :],
                        s4[:, 0:64],
                        w4_t[H][zg][:, off : off + 512],
                        start=True,
                        stop=True,
                        tile_position=(0, H * 64),
                        skip_group_check=True,
                    )
                xz = work.tile([128, 512], DT, tag="xz")
                nc.vector.tensor_mul(xz[:], z_ps[:], x8_t[zg][:, off : off + 512])
                w4 = wpool.tile([128, 512], DT, tag="w")
                for k4 in range(4):
                    d_ps = ps_d.tile([128, 16], F32, tag="d")
                    nc.tensor.matmul(
                        d_ps[:],
                        xz[:, k4 * 128 : k4 * 128 + 128],
                        oneD[:],
                        start=True,
                        stop=True,
                    )
                    # sigmoid with i-replication baked in: w[n, (i,b)] bf16
                    nc.scalar.activation(
                        w4[:, k4 * 128 : k4 * 128 + 128].rearrange(
                            "p (i b) -> p i b", i=8
                        ),
                        d_ps[:].unsqueeze(1).broadcast_to([128, 8, 16]),
                        AF.Sigmoid,
                    )
                wtiles.append(w4)

            # y = w * x: one flat [128, 512] bf16 mul per z-tile
            ytiles = {}

            def y_for_chunk(c):
                t = c // 4
                if t not in ytiles:
                    y4 = work.tile([128, 512], DT, tag="y")
                    nc.vector.tensor_mul(y4[:], xs_slice(4 * t, 512), wtiles[t][:])
                    ytiles[t] = y4
                return ytiles[t][:, (c % 4) * 128 : (c % 4) * 128 + 128]

            return s_sweep(y_for_chunk, f"i{it}")

        # ---- iteration 1 ----
        t1_sb = routing_pass(v0, 1)
        t1g = all_reduce(t1_sb, 1)
        s1 = small.tile([16, 32], F32, tag="s1")
        nc.vector.tensor_copy(s1[:, 0:16], t1g[:, 0:16])
        nc.vector.tensor_sub(s1[:, 16:32], st0g[:, 16:32], t1g[:, 16:32])
        v1 = squash(s1, 1.0)
        vacc2 = small.tile([16, 32], F32, tag="vacc2")
        nc.vector.tensor_add(vacc2[:], v0[:], v1[:])

        # ---- iteration 2 (partials out; host combines) ----
        t2_sb = routing_pass(vacc2, 2)
        nc.sync.dma_start(t2_out[:], t2_sb[:])
        nc.sync.dma_start(s0g_out[:], st0g[:])

    nc.compile()
    return nc


def _squash_np(s):
    sn = np.sum(s * s, axis=-1, keepdims=True)
    return sn / (1.0 + sn) / np.sqrt(sn + EPS) * s


def finish_host(results):
    """Combine per-core (t2, s0g) partials into v2 [16,2,16]."""
    t2 = sum(np.asarray(r["t2"], dtype=np.float64) for r in results)
    s0g = np.asarray(results[0]["s0g"], dtype=np.float64)
    s2 = np.empty((16, 2, 16), dtype=np.float64)
    s2[:, 0, :] = t2[:, 0:16]
    s2[:, 1, :] = s0g[:, 16:32] - t2[:, 16:32]
    return _squash_np(s2).astype(np.float32)


def run(x, W, **spmd_kwargs):
    from concourse.bass_utils import run_bass_kernel_spmd

    x = np.asarray(x, dtype=np.float32)
    W = np.asarray(W, dtype=np.float32)
    in_maps, ncp = host_prep(x, W)
    key = ("nc", ncp)
    if key not in _CACHE:
        _CACHE[key] = build_kernel(ncp)
    nc = _CACHE[key]
    res = run_bass_kernel_spmd(nc, in_maps, list(range(N_CORES)), **spmd_kwargs)
    return finish_host(res.results), res


def kernel(x, W):
    return run(x, W)[0]



# revision 10
# speedup vs baseline: 1.0407x; 1.0407x over previous
"""CapsuleLayer (dynamic routing) Trainium2 kernel, v2.

Math (see reference): u_hat[b,j,n,o] = sum_i x[b,n,i] W[j,n,i,o]; 3 routing
iterations of softmax-over-j (j=2 -> sigmoid of logit diff) + squash.

Design (n sharded over 8 cores, 90 chunks of 128 n per core):
  - s-sums via chunk-diagonal matmuls: lhsT = y-chunk [128n, (i,b)=128]
    (FWL-width stationary), rhs = Ws-chunk [128n, (i',jo)=256]; the
    [128,256] PSUM accumulates all 90 chunks; only the 8 diagonal
    [16b, 32jo] blocks are real -> summed on DVE.
  - logit pass per 512-n tile: z via 2 matmuls (lhsT = I4 (x) vT bf16,
    rhs = W4 fp8-e3m4), xz on DVE (x8 fp8), d via oneD matmuls into a
    [128,64] PSUM, one batched Sigmoid (ACT) -> w [128n,(k,b)] bf16,
    y = w*x via one broadcast DVE mul per tile.
  - AllReduce via CC stream on the gpsimd queue, which carries no bulk
    DMA; bulk loads ride sync/scalar/vector queues.
Final v2 = squash(s2) is computed on the host from per-core partials.
"""
import sys

sys.path.insert(0, "/opt/trn_rl_repo")

import numpy as np
import ml_dtypes

BF16 = ml_dtypes.bfloat16
FP8 = ml_dtypes.float8_e3m4  # TRN float8e3: 4 mantissa bits, max ~15.5
N_CORES = 8
B = 16
NIN = 91392
DI = 8
DO = 16
NC_N = NIN // N_CORES  # 11424
CHUNKS = 90
NCP = CHUNKS * 128  # 11520
GS = 8  # XW group size (chunks)
EPS = 1e-7

# z tiles: 22 of 512 cols + 1 of 256
ZT = [(t * 512, 512) for t in range(22)] + [(22 * 512, 256)]

_CACHE = {}


def host_prep(x, W, n_cores=N_CORES):
    n_per = x.shape[1] // n_cores  # 11424
    oneD = np.zeros((128, 16), dtype=BF16)
    for i in range(DI):
        for b in range(B):
            oneD[i * 16 + b, b] = 1.0
    in_maps = []
    for c in range(n_cores):
        xc = np.zeros((B, NCP, DI), dtype=np.float32)
        Wc = np.zeros((2, NCP, DI, DO), dtype=np.float32)
        xc[:, :n_per] = x[:, c * n_per : (c + 1) * n_per]
        Wc[:, :n_per] = W[:, c * n_per : (c + 1) * n_per]
        # xs[n128, c, (i,b)] ; ws[n128, c, (i,(j,o))]
        xs = (
            xc.reshape(B, CHUNKS, 128, DI).transpose(2, 1, 3, 0).reshape(128, CHUNKS, 128)
        ).astype(BF16)
        ws = (
            Wc.reshape(2, CHUNKS, 128, DI, DO)
            .transpose(2, 1, 3, 0, 4)
            .reshape(128, CHUNKS, 256)
        ).astype(BF16)
        # XW groups of GS chunks: [xs_g | ws_g]
        cols = []
        for g0 in range(0, CHUNKS, GS):
            g1 = min(g0 + GS, CHUNKS)
            cols.append(xs[:, g0:g1].reshape(128, -1))
            cols.append(ws[:, g0:g1].reshape(128, -1))
        XW = np.ascontiguousarray(np.concatenate(cols, axis=1))
        # W4[(il,j,o), (H, n)] = W[j, n, H*4+il, o]
        W4 = np.ascontiguousarray(
            Wc.reshape(2, NCP, 2, 4, DO).transpose(3, 0, 4, 2, 1).reshape(128, 2 * NCP)
        ).astype(FP8)
        # x8[(i,b), n] = x[b,n,i]
        x8 = np.ascontiguousarray(xc.transpose(2, 0, 1).reshape(128, NCP)).astype(FP8)
        in_maps.append({"XW": XW, "W4": W4, "x8": x8, "oneD": oneD})
    return in_maps


def build_kernel(num_devices=N_CORES):
    from contextlib import ExitStack

    import concourse.bacc as bacc
    import concourse.tile as tile
    from concourse import mybir

    DT = mybir.dt.bfloat16
    F8 = mybir.dt.float8e3
    F32 = mybir.dt.float32
    AF = mybir.ActivationFunctionType

    ngrp = (CHUNKS + GS - 1) // GS  # 12 (last group has 2 chunks)
    xw_cols = CHUNKS * 384  # 34560

    nc = bacc.Bacc(
        "TRN2", target_bir_lowering=False, debug=False, num_devices=num_devices
    )
    xw_in = nc.declare_dram_parameter("XW", [128, xw_cols], DT, isOutput=False)
    w4_in = nc.declare_dram_parameter("W4", [128, 2 * NCP], F8, isOutput=False)
    x8_in = nc.declare_dram_parameter("x8", [128, NCP], F8, isOutput=False)
    oned_in = nc.declare_dram_parameter("oneD", [128, 16], DT, isOutput=False)
    t2_out = nc.declare_dram_parameter("t2", [32, 64], F32, isOutput=True)
    s0g_out = nc.declare_dram_parameter("s0g", [16, 32], F32, isOutput=True)

    ar_bufs = []
    for k in range(3):
        ar_bufs.append(
            (
                nc.dram_tensor(f"ar_in{k}", [32, 64], F32),
                nc.dram_tensor(f"ar_out{k}", [32, 64], F32, addr_space="Shared"),
            )
        )

    # W4 park slices by z-tile groups (4,4,4,4,4,3 tiles)
    w4_slices = []  # (n0, ncols)
    for s in range(5):
        w4_slices.append((s * 2048, 2048))
    w4_slices.append((10240, NCP - 10240))  # 1280

    with tile.TileContext(nc) as tc, ExitStack() as ctx:
        park = ctx.enter_context(tc.tile_pool(name="park", bufs=1))
        ps_acc = ctx.enter_context(tc.tile_pool(name="ps_acc", bufs=1, space="PSUM"))
        ps_z = ctx.enter_context(tc.tile_pool(name="ps_z", bufs=2, space="PSUM"))
        ps_d = ctx.enter_context(tc.tile_pool(name="ps_d", bufs=2, space="PSUM"))
        work = ctx.enter_context(tc.tile_pool(name="work", bufs=3))
        ypool = ctx.enter_context(tc.tile_pool(name="ypool", bufs=len(ZT)))
        small = ctx.enter_context(tc.tile_pool(name="small", bufs=8))

        # ---- pre-sync AllReduce: warms the CC stream + absorbs launch skew.
        pre_in, pre_out = ar_bufs[2]
        zt_sb = work.tile([32, 64], F32, tag="zt_sb")
        nc.gpsimd.memset(zt_sb[:], 0.0)
        nc.gpsimd.dma_start(pre_in[:], zt_sb[:])
        nc.gpsimd.collective_compute(
            "AllReduce",
            mybir.AluOpType.add,
            replica_groups=[list(range(num_devices))],
            ins=[pre_in[:]],
            outs=[pre_out[:]],
        )

        # ---- bulk loads ----
        # XW (needed before AR0) rides sync/scalar/gpsimd; gpsimd's share
        # drains before the AR0 bounce needs that queue. W4/x8 (needed
        # after AR0, slice-paced by pass 1) ride sync/scalar only.
        xw_engs = [nc.sync, nc.scalar, nc.gpsimd]
        _rr = [0]

        def load_xw(dst_ap, src_ap):
            xw_engs[_rr[0] % 3].dma_start(dst_ap, src_ap)
            _rr[0] += 1

        xw_t = []
        off = 0
        for g in range(ngrp):
            gsz = min(GS, CHUNKS - g * GS)
            w = gsz * 384
            t = park.tile([128, w], DT, tag=f"xw{g}")
            load_xw(t[:], xw_in[:, off : off + w])
            xw_t.append((t, gsz))
            off += w
        oneD = park.tile([128, 16], DT, tag="oneD")
        nc.sync.dma_start(oneD[:], oned_in[:])
        w4_view = w4_in[:].rearrange("p (h n) -> p h n", h=2)
        w4_t = []
        x8_t = []
        for k, (n0, ncols) in enumerate(w4_slices):
            eng = nc.sync if k % 2 == 0 else nc.scalar
            t = park.tile([128, 2 * ncols], F8, tag=f"w4_{k}")
            eng.dma_start(
                t[:].rearrange("p (h n) -> p h n", h=2),
                w4_view[:, :, n0 : n0 + ncols],
            )
            w4_t.append(t)
            tx = park.tile([128, ncols], F8, tag=f"x8_{k}")
            eng.dma_start(tx[:], x8_in[:, n0 : n0 + ncols])
            x8_t.append(tx)

        def xs_chunk(c):
            g, lc = c // GS, c % GS
            t, gsz = xw_t[g]
            return t[:, lc * 128 : lc * 128 + 128]

        def xs_cols(c, w):
            g, lc = c // GS, c % GS
            t, gsz = xw_t[g]
            return t[:, lc * 128 : lc * 128 + w]

        def ws_chunk(c):
            g, lc = c // GS, c % GS
            t, gsz = xw_t[g]
            off = gsz * 128 + lc * 256
            return t[:, off : off + 256]

        def w4_slice(H, n0, ncols):
            s = n0 // 2048
            t = w4_t[s]
            loc = n0 - w4_slices[s][0]
            sc = w4_slices[s][1]
            return t[:].rearrange("p (h n) -> p h n", h=2)[:, H, loc : loc + ncols]

        def x8_slice(n0, ncols):
            s = n0 // 2048
            loc = n0 - w4_slices[s][0]
            return x8_t[s][:, loc : loc + ncols]

        def diag_extract(ps, tag):
            """Sum the diagonal of a [128,256] sweep PSUM into a [32,64] slab.

            Engine APs must start at 32-aligned partitions, so sum the four
            [32,64] diagonal slabs; the true [16,32] total is
            slab[0:16,0:32] + slab[16:32,32:64] (folded post-AllReduce via
            DRAM reads, or on the host for the final sweep).
            """
            a = small.tile([32, 64], F32, tag=f"dxa0_{tag}")
            nc.vector.tensor_copy(a[:], ps[0:32, 0:64])
            for q in range(1, 4):
                nxt = small.tile([32, 64], F32, tag=f"dxa{q}_{tag}")
                nc.vector.tensor_add(
                    nxt[:], a[:], ps[q * 32 : q * 32 + 32, q * 64 : q * 64 + 64]
                )
                a = nxt
            return a

        def squash(s_tile, scale):
            """v = squash(scale * s), s_tile [16,32] viewed [16,2,16]."""
            sq = small.tile([16, 32], F32, tag="sq")
            nc.vector.tensor_mul(sq[:], s_tile[:], s_tile[:])
            sn = small.tile([16, 2], F32, tag="sn")
            nc.vector.tensor_reduce(
                sn[:],
                sq[:].rearrange("p (j o) -> p j o", j=2),
                mybir.AxisListType.X,
                mybir.AluOpType.add,
            )
            sns = small.tile([16, 2], F32, tag="sns")
            nc.vector.tensor_scalar_mul(sns[:], sn[:], scale * scale)
            den = small.tile([16, 2], F32, tag="den")
            nc.vector.tensor_scalar_add(den, sns[:], 1.0)
            rec = small.tile([16, 2], F32, tag="rec")
            nc.vector.reciprocal(rec[:], den[:])
            epst = small.tile([16, 1], F32, tag="epst")
            nc.vector.memset(epst[:], EPS)
            sr = small.tile([16, 2], F32, tag="sr")
            nc.scalar.activation(sr[:], sns[:], AF.Sqrt, bias=epst[:])
            rs = small.tile([16, 2], F32, tag="rs")
            nc.vector.reciprocal(rs[:], sr[:])
            f = small.tile([16, 2], F32, tag="f")
            nc.vector.tensor_mul(f[:], sns[:], rec[:])
            f2 = small.tile([16, 2], F32, tag="f2")
            nc.vector.tensor_mul(f2[:], f[:], rs[:])
            fs = small.tile([16, 2], F32, tag="fs")
            nc.vector.tensor_scalar_mul(fs[:], f2[:], scale)
            v = small.tile([16, 32], F32, tag=f"v_{scale}_{nc.next_id()}")
            nc.vector.tensor_mul(
                v[:].rearrange("p (j o) -> p j o", j=2),
                s_tile[:].rearrange("p (j o) -> p j o", j=2),
                fs[:].unsqueeze(2).broadcast_to([16, 2, 16]),
            )
            return v

        def all_reduce(src_slab, idx):
            """SBUF [32,64] slab partial -> SBUF [16,32] folded global sum."""
            a_in, a_out = ar_bufs[idx]
            nc.gpsimd.dma_start(a_in[:], src_slab[:])
            nc.gpsimd.collective_compute(
                "AllReduce",
                mybir.AluOpType.add,
                replica_groups=[list(range(num_devices))],
                ins=[a_in[:]],
                outs=[a_out[:]],
            )
            g1 = small.tile([16, 32], F32, tag=f"arg1_{idx}")
            g2 = small.tile([16, 32], F32, tag=f"arg2_{idx}")
            nc.gpsimd.dma_start(g1[:], a_out[0:16, 0:32])
            nc.gpsimd.dma_start(g2[:], a_out[16:32, 32:64])
            g = small.tile([16, 32], F32, tag=f"arg{idx}")
            nc.vector.tensor_add(g[:], g1[:], g2[:])
            return g

        # ---- stage A: t0[b,(j,o)] = sum_{n,i} x W (chunk-diagonal sweep) ----
        stA = ps_acc.tile([128, 256], F32, tag="stA")
        for c in range(CHUNKS):
            nc.tensor.matmul(
                stA[:],
                xs_chunk(c),
                ws_chunk(c),
                start=(c == 0),
                stop=(c == CHUNKS - 1),
                skip_group_check=True,
            )
        t0p = diag_extract(stA, "a")
        t0g = all_reduce(t0p, 0)
        v0 = squash(t0g, 0.5)

        def routing_pass(vacc, it):
            # vT[(j,o), b] with sign: +v[j=0], -v[j=1]
            vt_in = work.tile([32, 32], F32, tag="vt_in")
            nc.vector.memset(vt_in[:], 0.0)
            nc.vector.tensor_copy(vt_in[0:16, 0:16], vacc[:, 0:16])
            nc.scalar.mul(vt_in[0:16, 16:32], vacc[:, 16:32], -1.0)
            vT = work.tile([32, 32], F32, tag="vT")
            nc.vector.transpose(vT[:], vt_in[:])
            # S4 = I_4 (x) vT : [128 (il,j,o), 64 (g,b)] bf16
            s4 = work.tile([128, 64], DT, tag="s4")
            nc.vector.memset(s4[:], 0.0)
            for gg in range(4):
                nc.scalar.copy(
                    s4[gg * 32 : gg * 32 + 32, gg * 16 : gg * 16 + 16],
                    vT[0:32, 0:16],
                )

            ytiles = []
            for ti, (n0, nt) in enumerate(ZT):
                nk = nt // 128
                z_ps = ps_z.tile([128, 512], F32, tag="z")
                for H in (0, 1):
                    nc.tensor.matmul(
                        z_ps[H * 64 : H * 64 + 64, :nt],
                        s4[:, 0:64],
                        w4_slice(H, n0, nt),
                        start=True,
                        stop=True,
                        tile_position=(0, H * 64),
                        skip_group_check=True,
                    )
                xz = work.tile([128, 512], DT, tag="xz")
                nc.vector.tensor_mul(xz[:, :nt], z_ps[:, :nt], x8_slice(n0, nt))
                d_ps = ps_d.tile([128, 64], F32, tag="d")
                for k in range(nk):
                    nc.tensor.matmul(
                        d_ps[:, k * 16 : k * 16 + 16],
                        xz[:, k * 128 : k * 128 + 128],
                        oneD[:],
                        start=True,
                        stop=True,
                        skip_group_check=True,
                    )
                w4b = work.tile([128, 64], DT, tag="w4b")
                nc.scalar.activation(
                    w4b[:, : nk * 16], d_ps[:, : nk * 16], AF.Sigmoid
                )
                y = ypool.tile([128, 512], DT, tag="y")
                nc.vector.tensor_mul(
                    y[:, :nt].rearrange("p (k i b) -> p k i b", k=nk, i=8),
                    xs_cols(4 * ti, nt).rearrange("p (k i b) -> p k i b", k=nk, i=8),
                    w4b[:, : nk * 16]
                    .rearrange("p (k b) -> p k b", k=nk)
                    .unsqueeze(2)
                    .broadcast_to([128, nk, 8, 16]),
                )
                ytiles.append(y)

            stP = ps_acc.tile([128, 256], F32, tag=f"stP{it}")
            for c in range(CHUNKS):
                y = ytiles[c // 4]
                nc.tensor.matmul(
                    stP[:],
                    y[:, (c % 4) * 128 : (c % 4) * 128 + 128],
                    ws_chunk(c),
                    start=(c == 0),
                    stop=(c == CHUNKS - 1),
                    skip_group_check=True,
                )
            return diag_extract(stP, f"i{it}")

        # ---- iteration 1 ----
        t1p = routing_pass(v0, 1)
        t1g = all_reduce(t1p, 1)
        s1 = small.tile([16, 32], F32, tag="s1")
        nc.vector.tensor_copy(s1[:, 0:16], t1g[:, 0:16])
        nc.vector.tensor_sub(s1[:, 16:32], t0g[:, 16:32], t1g[:, 16:32])
        v1 = squash(s1, 1.0)
        vacc2 = small.tile([16, 32], F32, tag="vacc2")
        nc.vector.tensor_add(vacc2[:], v0[:], v1[:])

        # ---- iteration 2 (partials out; host combines) ----
        t2p = routing_pass(vacc2, 2)
        nc.sync.dma_start(t2_out[:], t2p[:])
        nc.sync.dma_start(s0g_out[:], t0g[:])

    nc.compile()
    return nc


def _squash_np(s):
    sn = np.sum(s * s, axis=-1, keepdims=True)
    return sn / (1.0 + sn) / np.sqrt(sn + EPS) * s


def finish_host(results):
    """Combine per-core (t2 slab, s0g) partials into v2 [16,2,16]."""
    t2s = sum(np.asarray(r["t2"], dtype=np.float64) for r in results)
    t2 = t2s[0:16, 0:32] + t2s[16:32, 32:64]
    s0g = np.asarray(results[0]["s0g"], dtype=np.float64)
    s2 = np.empty((16, 2, 16), dtype=np.float64)
    s2[:, 0, :] = t2[:, 0:16]
    s2[:, 1, :] = s0g[:, 16:32] - t2[:, 16:32]
    return _squash_np(s2).astype(np.float32)


def run(x, W, **spmd_kwargs):
    from concourse.bass_utils import run_bass_kernel_spmd

    x = np.asarray(x, dtype=np.float32)
    W = np.asarray(W, dtype=np.float32)
    in_maps = host_prep(x, W)
    key = "nc_v2"
    if key not in _CACHE:
        _CACHE[key] = build_kernel()
    nc = _CACHE[key]
    res = run_bass_kernel_spmd(nc, in_maps, list(range(N_CORES)), **spmd_kwargs)
    return finish_host(res.results), res


def kernel(x, W):
    return run(x, W)[0]


# revision 17
# speedup vs baseline: 1.2789x; 1.2290x over previous
"""CapsuleLayer (dynamic routing) Trainium2 kernel, v3.

Math (see reference): u_hat[b,j,n,o] = sum_i x[b,n,i] W[j,n,i,o]; 3 routing
iterations of softmax-over-j (j=2 -> sigmoid of logit diff) + squash.

Design (n sharded over 8 cores, 90 chunks of 128 n per core):
  - s-sums via chunk-diagonal matmuls, split M=64 with alternating
    tile_position columns (0,0)/(0,64) so LDWEIGHTS of one half overlaps
    the other half's MATMUL. PSUM [128,128]; diagonal [16,32] blocks are
    summed as four 32-aligned [32,64] slabs -> [32,64] partial; the
    odd/even fold happens after the AllReduce (via DRAM reads) or on the
    host for the final sweep.
  - logit pass per 512-n tile: z via 2 matmuls (lhsT = I4 (x) vT bf16,
    rhs = W4 fp8-e3m4), xz on DVE (x8 fp8), d via oneD matmuls (M=64
    alternating), one batched Sigmoid (ACT) -> w [128n,(k,b)] bf16,
    y = w*x via one broadcast mul per tile (DVE, some tiles on GpSimd).
  - squash computed entirely on DVE (quake rsqrt + 2 Newton steps; v only
    feeds routing logits, the final v2 is squashed on the host), so the
    ACT sigmoid table never gets swapped out.
  - AllReduce via CC stream; its bounce DMAs ride the sync queue which
    carries only early XW loads. W4/x8 ride scalar/gpsimd queues.
"""
import sys

sys.path.insert(0, "/opt/trn_rl_repo")

import numpy as np
import ml_dtypes

BF16 = ml_dtypes.bfloat16
FP8 = ml_dtypes.float8_e3m4  # TRN float8e3: 4 mantissa bits, max ~15.5
N_CORES = 8
B = 16
NIN = 91392
DI = 8
DO = 16
NC_N = NIN // N_CORES  # 11424
CHUNKS = 90
NCP = CHUNKS * 128  # 11520
GS = 8  # XW group size (chunks)
EPS = 1e-7

# z tiles: 22 of 512 cols + 1 of 256
ZT = [(t * 512, 512) for t in range(22)] + [(22 * 512, 256)]

_CACHE = {}


def _patch_walrus_flags():
    """No-op: walrus' ldw-opt rejects bass-emitted InstLdweights."""


def host_prep(x, W, n_cores=N_CORES):
    n_per = x.shape[1] // n_cores  # 11424
    oneD = np.zeros((128, 16), dtype=BF16)
    for i in range(DI):
        for b in range(B):
            oneD[i * 16 + b, b] = 1.0
    in_maps = []
    for c in range(n_cores):
        xc = np.zeros((B, NCP, DI), dtype=np.float32)
        Wc = np.zeros((2, NCP, DI, DO), dtype=np.float32)
        xc[:, :n_per] = x[:, c * n_per : (c + 1) * n_per]
        Wc[:, :n_per] = W[:, c * n_per : (c + 1) * n_per]
        # xs[n128, c, (i,b)] ; ws[n128, c, (i,(j,o))]
        xs = (
            xc.reshape(B, CHUNKS, 128, DI).transpose(2, 1, 3, 0).reshape(128, CHUNKS, 128)
        ).astype(BF16)
        ws = (
            Wc.reshape(2, CHUNKS, 128, DI, DO)
            .transpose(2, 1, 3, 0, 4)
            .reshape(128, CHUNKS, 256)
        ).astype(BF16)
        cols = []
        for g0 in range(0, CHUNKS, GS):
            g1 = min(g0 + GS, CHUNKS)
            cols.append(xs[:, g0:g1].reshape(128, -1))
            cols.append(ws[:, g0:g1].reshape(128, -1))
        XW = np.ascontiguousarray(np.concatenate(cols, axis=1))
        # W4[(il,j,o), (H, n)] = W[j, n, H*4+il, o]
        W4 = np.ascontiguousarray(
            Wc.reshape(2, NCP, 2, 4, DO).transpose(3, 0, 4, 2, 1).reshape(128, 2 * NCP)
        ).astype(FP8)
        # x8[(i,b), n] = x[b,n,i]
        x8 = np.ascontiguousarray(xc.transpose(2, 0, 1).reshape(128, NCP)).astype(FP8)
        in_maps.append({"XW": XW, "W4": W4, "x8": x8, "oneD": oneD})
    return in_maps


def build_kernel(num_devices=N_CORES):
    from contextlib import ExitStack

    import concourse.bacc as bacc
    import concourse.tile as tile
    from concourse import mybir

    DT = mybir.dt.bfloat16
    F8 = mybir.dt.float8e3
    F32 = mybir.dt.float32
    U32 = mybir.dt.uint32
    AF = mybir.ActivationFunctionType
    OP = mybir.AluOpType

    ngrp = (CHUNKS + GS - 1) // GS  # 12 (last group has 2 chunks)
    xw_cols = CHUNKS * 384  # 34560

    nc = bacc.Bacc(
        "TRN2", target_bir_lowering=False, debug=False, num_devices=num_devices
    )
    xw_in = nc.declare_dram_parameter("XW", [128, xw_cols], DT, isOutput=False)
    w4_in = nc.declare_dram_parameter("W4", [128, 2 * NCP], F8, isOutput=False)
    x8_in = nc.declare_dram_parameter("x8", [128, NCP], F8, isOutput=False)
    oned_in = nc.declare_dram_parameter("oneD", [128, 16], DT, isOutput=False)
    t2_out = nc.declare_dram_parameter("t2", [32, 64], F32, isOutput=True)
    s0g_out = nc.declare_dram_parameter("s0g", [16, 32], F32, isOutput=True)

    ar_bufs = []
    for k in range(3):
        ar_bufs.append(
            (
                nc.dram_tensor(f"ar_in{k}", [32, 64], F32),
                nc.dram_tensor(f"ar_out{k}", [32, 64], F32, addr_space="Shared"),
            )
        )

    # W4/x8 park slices by z-tile groups (4,4,4,4,4,3 tiles)
    w4_slices = [(s * 2048, 2048) for s in range(5)] + [(10240, NCP - 10240)]

    with tile.TileContext(nc) as tc, ExitStack() as ctx:
        park = ctx.enter_context(tc.tile_pool(name="park", bufs=1))
        ps_acc = ctx.enter_context(tc.tile_pool(name="ps_acc", bufs=1, space="PSUM"))
        ps_z = ctx.enter_context(tc.tile_pool(name="ps_z", bufs=2, space="PSUM"))
        ps_d = ctx.enter_context(tc.tile_pool(name="ps_d", bufs=2, space="PSUM"))
        work = ctx.enter_context(tc.tile_pool(name="work", bufs=3))
        ypool = ctx.enter_context(tc.tile_pool(name="ypool", bufs=len(ZT)))
        small = ctx.enter_context(tc.tile_pool(name="small", bufs=8))

        # ---- pre-sync AllReduce: warms the CC stream + absorbs launch skew.
        pre_in, pre_out = ar_bufs[2]
        zt_sb = work.tile([32, 64], F32, tag="zt_sb")
        nc.gpsimd.memset(zt_sb[:], 0.0)
        nc.gpsimd.dma_start(pre_in[:], zt_sb[:])
        nc.gpsimd.collective_compute(
            "AllReduce",
            mybir.AluOpType.add,
            replica_groups=[list(range(num_devices))],
            ins=[pre_in[:]],
            outs=[pre_out[:]],
        )

        # ---- bulk loads ----
        # sync carries only XW (drains early) so the AR bounce DMAs queued
        # on it later fire promptly; scalar/gpsimd carry the W4/x8 tail,
        # consumed slice-paced by pass 1 well after AR0.
        xw_engs = [nc.sync, nc.scalar, nc.gpsimd]
        xw_t = []
        off = 0
        for g in range(ngrp):
            gsz = min(GS, CHUNKS - g * GS)
            w = gsz * 384
            t = park.tile([128, w], DT, tag=f"xw{g}")
            xw_engs[g % 3].dma_start(t[:], xw_in[:, off : off + w])
            xw_t.append((t, gsz))
            off += w
        oneD = park.tile([128, 16], DT, tag="oneD")
        nc.sync.dma_start(oneD[:], oned_in[:])
        w4_view = w4_in[:].rearrange("p (h n) -> p h n", h=2)
        w4_t = []
        x8_t = []
        for k, (n0, ncols) in enumerate(w4_slices):
            eng = nc.scalar if k % 2 == 0 else nc.gpsimd
            t = park.tile([128, 2 * ncols], F8, tag=f"w4_{k}")
            eng.dma_start(
                t[:].rearrange("p (h n) -> p h n", h=2),
                w4_view[:, :, n0 : n0 + ncols],
            )
            w4_t.append(t)
            tx = park.tile([128, ncols], F8, tag=f"x8_{k}")
            eng.dma_start(tx[:], x8_in[:, n0 : n0 + ncols])
            x8_t.append(tx)

        # pre-zeroed skeletons for the per-pass stationaries
        vtp = park.tile([32, 32], F32, tag="vtp")
        nc.vector.memset(vtp[:], 0.0)
        s4sk = {}
        for it in (1, 2):
            s4t = park.tile([128, 64], DT, tag=f"s4_{it}", name=f"s4_{it}")
            nc.gpsimd.memset(s4t[:], 0.0)
            s4sk[it] = s4t
        # per-(b,j) sign/scale columns for baking the j=1 minus sign into v
        sgn1 = park.tile([16, 2], F32, tag="sgn1")  # [0.5, -0.5] (stage A)
        sgn2 = park.tile([16, 2], F32, tag="sgn2")  # [1, -1]
        nc.vector.memset(sgn1[:, 0:1], 0.5)
        nc.vector.memset(sgn1[:, 1:2], -0.5)
        nc.vector.memset(sgn2[:, 0:1], 1.0)
        nc.vector.memset(sgn2[:, 1:2], -1.0)

        def xs_cols(c, w):
            g, lc = c // GS, c % GS
            t, gsz = xw_t[g]
            return t[:, lc * 128 : lc * 128 + w]

        def ws_chunk(c):
            g, lc = c // GS, c % GS
            t, gsz = xw_t[g]
            off = gsz * 128 + lc * 256
            return t[:, off : off + 256]

        def w4_slice(H, n0, ncols):
            s = n0 // 2048
            loc = n0 - w4_slices[s][0]
            return w4_t[s][:].rearrange("p (h n) -> p h n", h=2)[
                :, H, loc : loc + ncols
            ]

        def x8_slice(n0, ncols):
            s = n0 // 2048
            loc = n0 - w4_slices[s][0]
            return x8_t[s][:, loc : loc + ncols]

        def sweep_mms(ps, c, lhs128, last):
            """Two M=64 chunk-diagonal matmuls with alternating columns."""
            for h in (0, 1):
                nc.tensor.matmul(
                    ps[h * 64 : h * 64 + 64, :],
                    lhs128[:, h * 64 : h * 64 + 64],
                    ws_chunk(c)[:, h * 128 : h * 128 + 128],
                    start=(c == 0),
                    stop=last,
                    tile_position=(0, h * 64),
                    skip_group_check=True,
                )

        def diag_extract(ps, tag):
            """[128,128] sweep PSUM -> [32,64] slab (true total needs the
            odd/even fold: slab[0:16,0:32] + slab[16:32,32:64])."""
            slabs = [
                ps[0:32, 0:64],
                ps[32:64, 64:128],
                ps[64:96, 0:64],
                ps[96:128, 64:128],
            ]
            a = small.tile([32, 64], F32, tag=f"dxa0_{tag}")
            nc.vector.tensor_copy(a[:], slabs[0])
            for q in range(1, 4):
                nxt = small.tile([32, 64], F32, tag=f"dxa{q}_{tag}")
                nc.vector.tensor_add(nxt[:], a[:], slabs[q])
                a = nxt
            return a

        def squash_pm(s_tile, sgn, tag):
            """vpm = squash(scale*s) with the j=1 sign flip baked in.

            sgn is a [16,2] per-(b,j) column of +/-scale. All on DVE: quake
            rsqrt + 2 Newton steps (v only feeds routing logits).
            """
            sq = small.tile([16, 32], F32, tag=f"sq_{tag}")
            nc.vector.tensor_mul(sq[:], s_tile[:], s_tile[:])
            sn = small.tile([16, 2], F32, tag=f"sn_{tag}")
            nc.vector.tensor_reduce(
                sn[:],
                sq[:].rearrange("p (j o) -> p j o", j=2),
                mybir.AxisListType.X,
                mybir.AluOpType.add,
            )
            # a = scale^2*sn + eps ; scale is baked via sgn (sgn^2 = scale^2)
            sc2 = small.tile([16, 2], F32, tag=f"sc2_{tag}")
            nc.vector.tensor_mul(sc2[:], sgn[:], sgn[:])
            sns = small.tile([16, 2], F32, tag=f"sns_{tag}")
            nc.vector.tensor_mul(sns[:], sn[:], sc2[:])
            # rs = 1/sqrt(sns+eps) via ACT Sqrt + DVE reciprocal (the
            # sqrt-table load hides in ACT idle time during the AllReduce)
            epst = small.tile([16, 1], F32, tag=f"epst_{tag}")
            nc.vector.memset(epst[:], EPS)
            sr = small.tile([16, 2], F32, tag=f"sr_{tag}")
            nc.scalar.activation(sr[:], sns[:], AF.Sqrt, bias=epst[:])
            rs = small.tile([16, 2], F32, tag=f"rs_{tag}")
            nc.vector.reciprocal(rs[:], sr[:])
            # f_pm = sgn * rsqrt * sns/(1+sns)
            den = small.tile([16, 2], F32, tag=f"den_{tag}")
            nc.vector.tensor_scalar_add(den[:], sns[:], 1.0)
            rec = small.tile([16, 2], F32, tag=f"rec_{tag}")
            nc.vector.reciprocal(rec[:], den[:])
            nr = small.tile([16, 2], F32, tag=f"nr_{tag}")
            nc.vector.tensor_mul(nr[:], sns[:], rec[:])
            f1 = small.tile([16, 2], F32, tag=f"f1_{tag}")
            nc.vector.tensor_mul(f1[:], rs[:], nr[:])
            fpm = small.tile([16, 2], F32, tag=f"fpm_{tag}")
            nc.vector.tensor_mul(fpm[:], f1[:], sgn[:])
            v = small.tile([16, 32], F32, tag=f"v_{tag}")
            nc.vector.tensor_mul(
                v[:].rearrange("p (j o) -> p j o", j=2),
                s_tile[:].rearrange("p (j o) -> p j o", j=2),
                fpm[:].unsqueeze(2).broadcast_to([16, 2, 16]),
            )
            return v

        def all_reduce(src_slab, idx):
            """SBUF [32,64] slab partial -> SBUF [16,32] folded global sum."""
            a_in, a_out = ar_bufs[idx]
            nc.sync.dma_start(a_in[:], src_slab[:])
            nc.gpsimd.collective_compute(
                "AllReduce",
                mybir.AluOpType.add,
                replica_groups=[list(range(num_devices))],
                ins=[a_in[:]],
                outs=[a_out[:]],
            )
            g1 = small.tile([16, 32], F32, tag=f"arg1_{idx}")
            g2 = small.tile([16, 32], F32, tag=f"arg2_{idx}")
            nc.sync.dma_start(g1[:], a_out[0:16, 0:32])
            nc.sync.dma_start(g2[:], a_out[16:32, 32:64])
            g = small.tile([16, 32], F32, tag=f"arg{idx}")
            nc.vector.tensor_add(g[:], g1[:], g2[:])
            return g

        # ---- stage A: t0[b,(j,o)] = sum_{n,i} x W (chunk-diagonal sweep) ----
        stA = ps_acc.tile([128, 128], F32, tag="stA")
        for c in range(CHUNKS):
            sweep_mms(stA, c, xs_cols(c, 128), last=(c == CHUNKS - 1))
        t0p = diag_extract(stA, "a")
        t0g = all_reduce(t0p, 0)
        vpm0 = squash_pm(t0g, sgn1, "v0")

        def routing_pass(vpm, it):
            # vT[(j,o), b] from the signed v via one copy + block transpose
            nc.vector.tensor_copy(vtp[0:16, :], vpm[:])
            vT = work.tile([32, 32], F32, tag="vT")
            nc.vector.transpose(vT[:], vtp[:])
            s4 = s4sk[it]
            for gg in range(4):
                nc.scalar.copy(
                    s4[gg * 32 : gg * 32 + 32, gg * 16 : gg * 16 + 16],
                    vT[0:32, 0:16],
                )

            ytiles = []
            for ti, (n0, nt) in enumerate(ZT):
                nk = nt // 128
                z_ps = ps_z.tile([128, 512], F32, tag="z")
                for H in (0, 1):
                    nc.tensor.matmul(
                        z_ps[H * 64 : H * 64 + 64, :nt],
                        s4[:, 0:64],
                        w4_slice(H, n0, nt),
                        start=True,
                        stop=True,
                        tile_position=(0, H * 64),
                        skip_group_check=True,
                    )
                xz = work.tile([128, 512], DT, tag="xz")
                nc.vector.tensor_mul(xz[:, :nt], z_ps[:, :nt], x8_slice(n0, nt))
                d_ps = ps_d.tile([128, 64], F32, tag="d")
                for k in range(nk):
                    for h in (0, 1):
                        nc.tensor.matmul(
                            d_ps[h * 64 : h * 64 + 64, k * 16 : k * 16 + 16],
                            xz[:, k * 128 + h * 64 : k * 128 + h * 64 + 64],
                            oneD[:],
                            start=True,
                            stop=True,
                            tile_position=(0, h * 64),
                            skip_group_check=True,
                        )
                w4b = work.tile([128, 64], DT, tag="w4b")
                nc.scalar.activation(
                    w4b[:, : nk * 16], d_ps[:, : nk * 16], AF.Sigmoid
                )
                y = ypool.tile([128, 512], DT, tag="y")
                nc.vector.tensor_mul(
                    y[:, :nt].rearrange("p (k i b) -> p k i b", k=nk, i=8),
                    xs_cols(4 * ti, nt).rearrange("p (k i b) -> p k i b", k=nk, i=8),
                    w4b[:, : nk * 16]
                    .rearrange("p (k b) -> p k b", k=nk)
                    .unsqueeze(2)
                    .broadcast_to([128, nk, 8, 16]),
                )
                ytiles.append(y)

            stP = ps_acc.tile([128, 128], F32, tag=f"stP{it}")
            for c in range(CHUNKS):
                y = ytiles[c // 4]
                sweep_mms(
                    stP,
                    c,
                    y[:, (c % 4) * 128 : (c % 4) * 128 + 128],
                    last=(c == CHUNKS - 1),
                )
            return diag_extract(stP, f"i{it}")

        # ---- iteration 1 ----
        t1p = routing_pass(vpm0, 1)
        t1g = all_reduce(t1p, 1)
        s1 = small.tile([16, 32], F32, tag="s1")
        nc.vector.tensor_copy(s1[:, 0:16], t1g[:, 0:16])
        nc.vector.tensor_sub(s1[:, 16:32], t0g[:, 16:32], t1g[:, 16:32])
        vpm1 = squash_pm(s1, sgn2, "v1")
        vacc2 = small.tile([16, 32], F32, tag="vacc2")
        nc.vector.tensor_add(vacc2[:], vpm0[:], vpm1[:])

        # ---- iteration 2 (partials out; host combines) ----
        t2p = routing_pass(vacc2, 2)
        nc.sync.dma_start(t2_out[:], t2p[:])
        nc.sync.dma_start(s0g_out[:], t0g[:])

    nc.compile()
    return nc


def _squash_np(s):
    sn = np.sum(s * s, axis=-1, keepdims=True)
    return sn / (1.0 + sn) / np.sqrt(sn + EPS) * s


def finish_host(results):
    """Combine per-core (t2 slab, s0g) partials into v2 [16,2,16]."""
    t2s = sum(np.asarray(r["t2"], dtype=np.float64) for r in results)
    t2 = t2s[0:16, 0:32] + t2s[16:32, 32:64]
    s0g = np.asarray(results[0]["s0g"], dtype=np.float64)
    s2 = np.empty((16, 2, 16), dtype=np.float64)
    s2[:, 0, :] = t2[:, 0:16]
    s2[:, 1, :] = s0g[:, 16:32] - t2[:, 16:32]
    return _squash_np(s2).astype(np.float32)


def run(x, W, **spmd_kwargs):
    from concourse.bass_utils import run_bass_kernel_spmd

    _patch_walrus_flags()
    x = np.asarray(x, dtype=np.float32)
    W = np.asarray(W, dtype=np.float32)
    in_maps = host_prep(x, W)
    key = "nc_v3"
    if key not in _CACHE:
        _CACHE[key] = build_kernel()
    nc = _CACHE[key]
    res = run_bass_kernel_spmd(nc, in_maps, list(range(N_CORES)), **spmd_kwargs)
    return finish_host(res.results), res


def kernel(x, W):
    return run(x, W)[0]
